# revision 20
# baseline (speedup 1.0000x reference)
"""Trainium2 Bass kernel for the 2-layer LSTMCell model.

Model (per timestep t, torch.nn.LSTMCell semantics, gates (i,f,g,o)):
    h0,c0 = LSTMCell(x_t, (h0,c0))   # D_IN=16  -> H1=100
    h1,c1 = LSTMCell(h0, (h1,c1))    # H1=100 -> H2=50
    y = h1_final @ W_fc.T + b_fc     # [B, 1]

Strategy (8 NeuronCores, data parallel over batch):
  - Each core handles B_local = 256 rows as 2 chunks of 128.
  - States are kept in transposed layout [H, B] in SBUF so they can feed the
    PE matmul as the stationary operand (contraction on partitions).
  - Gate matmul (mapping: batch on PSUM partitions, gates on free dim):
        gates0[128b, 400] = lhsT([h0T; ones; xT]).T @ W0aug[117, 400]
    with biases folded in via a constant ones-row that arrives with the x DMA.
  - Gate order is permuted to (i, f, o, g) so one Sigmoid instruction covers
    i,f,o contiguously and one Tanh covers g.
  - Elementwise work for both chunks is merged into single ACT/DVE
    instructions using 2-level access patterns across PSUM banks.
  - h_new is transposed back to [H, B] with PE transposes + a copy into the
    next step's stationary tile.
"""

import sys

import ml_dtypes
import numpy as np

BF = ml_dtypes.bfloat16

sys.path.insert(0, "/opt/trn_rl_repo")

import concourse.bacc as bacc
import concourse.bass as bass
import concourse.mybir as mybir
from concourse.tile import TileContext

F32 = mybir.dt.float32
F32R = mybir.dt.float32r
BF16 = mybir.dt.bfloat16
Act = mybir.ActivationFunctionType

B, T, D_IN = 2048, 2048, 16
H1, H2 = 100, 50
N_CORES = 8
B_LOCAL = B // N_CORES        # 256
NCH = 2                       # chunks of 128 per core

LAST_EXEC_NS = None
LAST_RESULTS = None

# ---------------------------------------------------------------- kernel build


def build_nc(t_steps=T):
    nc = bacc.Bacc("TRN2", target_bir_lowering=False)
    xt_d = nc.dram_tensor("xt", [t_steps + 1, 17, 256], F32R, kind="ExternalInput").ap()
    # all constants in one blob -> one DMA -> one sem for every weight use
    cb_d = nc.dram_tensor("cblob", [128, 1297], F32R, kind="ExternalInput").ap()
    a0_d = nc.dram_tensor("a0", [117, 256], F32R, kind="ExternalInput").ap()
    y_d = nc.dram_tensor("y", [256, 1], F32, kind="ExternalOutput").ap()

    with TileContext(nc) as tc:
        with (
            tc.tile_pool(name="consts", bufs=1) as cp,
            tc.tile_pool(name="apool", bufs=8) as apool,
            tc.tile_pool(name="bpool", bufs=8) as bpool,
            tc.tile_pool(name="ew", bufs=3) as ew,
            tc.tile_pool(name="gps", bufs=2, space="PSUM") as gps,
            tc.tile_pool(name="tps", bufs=1, space="PSUM") as tps,
        ):
            cb = cp.tile([128, 1041], F32R)  # zeros tail of cblob stays in DRAM
            nc.sync.dma_start(cb, cb_d[:, 0:1041])
            w0 = cb[0:117, 0:400]
            wih1 = cb[0:101, 400:656]
            whh1 = cb[0:50, 656:912]
            wfc = cb[0:51, 912:913]
            ident = cb[0:128, 913:1041]

            # initial state: A(0) fully from one DMA (zeros + ones row + x0);
            # the rest zeroed on DVE (one sem, later subsumed)
            A = apool.tile([117, 256], F32R, tag="A")
            nc.sync.dma_start(A, a0_d)
            Btile = bpool.tile([50, 256], F32R, tag="B")
            nc.sync.dma_start(Btile, cb_d[0:50, 1041:1297])
            c0 = ew.tile([128, 200], F32, tag="c0")
            nc.vector.memset(c0[:, :], 0.0)
            c1 = ew.tile([128, 100], F32, tag="c1")
            nc.vector.memset(c1[:, :], 0.0)

            for t in range(t_steps):
                # ---- layer 0 gates: [128b, 400] per chunk, 2 chunks in 2 banks
                g0 = gps.tile([128, 1024], F32, tag="g0")
                for c in range(NCH):
                    nc.tensor.matmul(
                        g0[:, c * 512 : c * 512 + 400],
                        A[:, c * 128 : (c + 1) * 128],
                        w0,
                        start=True,
                        stop=True,
                    )
                g0v = g0.rearrange("p (c f) -> p c f", c=2)  # [128, 2, 512]

                s0 = ew.tile([128, 600], F32, tag="s0")      # sig(i,f,o) both chunks
                s0v = s0.rearrange("p (c f) -> p c f", c=2)
                nc.scalar.activation(s0v, g0v[:, :, 0:300], Act.Sigmoid)
                tg0 = ew.tile([128, 200], F32, tag="tg0")    # tanh(g)
                tg0v = tg0.rearrange("p (c f) -> p c f", c=2)
                nc.scalar.activation(tg0v, g0v[:, :, 300:400], Act.Tanh)

                c0v = c0.rearrange("p (c f) -> p c f", c=2)
                m1 = ew.tile([128, 200], F32, tag="m1")
                m1v = m1.rearrange("p (c f) -> p c f", c=2)
                nc.vector.tensor_mul(m1v, s0v[:, :, 0:100], tg0v)
                m2 = ew.tile([128, 200], F32, tag="m2")
                m2v = m2.rearrange("p (c f) -> p c f", c=2)
                nc.vector.tensor_mul(m2v, s0v[:, :, 100:200], c0v)
                c0n = ew.tile([128, 200], F32, tag="c0")
                nc.vector.tensor_add(c0n, m1, m2)
                thc0 = ew.tile([128, 200], F32, tag="thc0")
                nc.scalar.activation(thc0, c0n, Act.Tanh)
                h0 = ew.tile([128, 200], F32R, tag="h0")
                h0v = h0.rearrange("p (c f) -> p c f", c=2)
                thc0v = thc0.rearrange("p (c f) -> p c f", c=2)
                nc.vector.tensor_mul(h0v, s0v[:, :, 200:300], thc0v)

                # ---- recycle h0 into the next stationary tile (transposed)
                An = apool.tile([117, 256], F32R, tag="A")
                nc.sync.dma_start(An[100:117, :], xt_d[t + 1])
                t0 = tps.tile([100, 256], F32R, tag="t0")
                for c in range(NCH):
                    nc.tensor.transpose(
                        t0[:, c * 128 : (c + 1) * 128],
                        h0[:, c * 100 : (c + 1) * 100],
                        ident,
                    )
                nc.vector.tensor_copy(An[0:100, :], t0)

                # ---- layer 1 gates: [128b, 200] per chunk, 1 bank
                g1 = gps.tile([128, 512], F32, tag="g1")
                for c in range(NCH):
                    nc.tensor.matmul(
                        g1[:, c * 256 : (c + 1) * 256],
                        An[0:101, c * 128 : (c + 1) * 128],
                        wih1,
                        start=True,
                        stop=False,
                    )
                    nc.tensor.matmul(
                        g1[:, c * 256 : (c + 1) * 256],
                        Btile[0:50, c * 128 : (c + 1) * 128],
                        whh1,
                        start=False,
                        stop=True,
                    )
                g1v = g1.rearrange("p (c f) -> p c f", c=2)  # [128, 2, 256]

                s1 = ew.tile([128, 300], F32, tag="s1")
                s1v = s1.rearrange("p (c f) -> p c f", c=2)
                nc.scalar.activation(s1v, g1v[:, :, 0:150], Act.Sigmoid)
                tg1 = ew.tile([128, 100], F32, tag="tg1")
                tg1v = tg1.rearrange("p (c f) -> p c f", c=2)
                nc.scalar.activation(tg1v, g1v[:, :, 150:200], Act.Tanh)

                c1v = c1.rearrange("p (c f) -> p c f", c=2)
                m3 = ew.tile([128, 100], F32, tag="m3")
                m3v = m3.rearrange("p (c f) -> p c f", c=2)
                nc.vector.tensor_mul(m3v, s1v[:, :, 0:50], tg1v)
                m4 = ew.tile([128, 100], F32, tag="m4")
                m4v = m4.rearrange("p (c f) -> p c f", c=2)
                nc.vector.tensor_mul(m4v, s1v[:, :, 50:100], c1v)
                c1n = ew.tile([128, 100], F32, tag="c1")
                nc.vector.tensor_add(c1n, m3, m4)
                thc1 = ew.tile([128, 100], F32, tag="thc1")
                nc.scalar.activation(thc1, c1n, Act.Tanh)
                h1 = ew.tile([128, 100], F32R, tag="h1")
                h1v = h1.rearrange("p (c f) -> p c f", c=2)
                thc1v = thc1.rearrange("p (c f) -> p c f", c=2)
                nc.vector.tensor_mul(h1v, s1v[:, :, 100:150], thc1v)

                Bn = bpool.tile([50, 256], F32R, tag="B")
                t1 = tps.tile([50, 256], F32R, tag="t1")
                for c in range(NCH):
                    nc.tensor.transpose(
                        t1[:, c * 128 : (c + 1) * 128],
                        h1[:, c * 50 : (c + 1) * 50],
                        ident,
                    )
                nc.scalar.copy(Bn, t1)

                A, Btile, c0, c1 = An, Bn, c0n, c1n

            # ---- final projection y = h1 @ W_fc.T + b_fc
            fin = ew.tile([51, 256], F32R, tag="fin")
            nc.vector.tensor_copy(fin[0:50, :], Btile)
            nc.sync.dma_start(fin[50:51, :], xt_d[t_steps, 0:1, :])
            yp = gps.tile([128, 2], F32, tag="g1")
            for c in range(NCH):
                nc.tensor.matmul(
                    yp[:, c : c + 1],
                    fin[:, c * 128 : (c + 1) * 128].bitcast(F32),
                    wfc.bitcast(F32),
                    start=True,
                    stop=True,
                )
            ysb = ew.tile([128, 2], F32, tag="ysb")
            nc.scalar.copy(ysb, yp)
            yv = y_d.rearrange("(c p) o -> c p o", c=2)
            for c in range(NCH):
                nc.sync.dma_start(yv[c], ysb[:, c : c + 1])
    return nc


# ---------------------------------------------------------------- host prep


def _gate_perm_rows(w, h):
    """Reorder gate rows (i,f,g,o) -> (i,f,o,g)."""
    return np.concatenate([w[0:h], w[h : 2 * h], w[3 * h : 4 * h], w[2 * h : 3 * h]], axis=0)


def prep_weights(W_ih0, W_hh0, b_ih0, b_hh0, W_ih1, W_hh1, b_ih1, b_hh1, W_fc, b_fc):
    """Pack all constants into one [128, 929] blob (single DMA)."""
    f32 = np.float32
    cb = np.zeros((128, 1297), f32)
    cb[0:100, 0:400] = _gate_perm_rows(np.asarray(W_hh0), H1).T
    cb[100, 0:400] = _gate_perm_rows(np.asarray(b_ih0 + b_hh0)[:, None], H1)[:, 0]
    cb[101:117, 0:400] = _gate_perm_rows(np.asarray(W_ih0), H1).T
    cb[0:100, 400:600] = _gate_perm_rows(np.asarray(W_ih1), H2).T
    cb[100, 400:600] = _gate_perm_rows(np.asarray(b_ih1 + b_hh1)[:, None], H2)[:, 0]
    cb[0:50, 656:856] = _gate_perm_rows(np.asarray(W_hh1), H2).T
    cb[0:50, 912] = np.asarray(W_fc)[0]
    cb[50, 912] = np.asarray(b_fc)[0]
    cb[:, 913:1041] = np.eye(128, dtype=f32)
    return cb


def prep_x_core(x_core, t_steps):
    """x_core [256, T, 16] -> [T+1, 17, 256] with ones row at index 0."""
    xt = np.empty((t_steps + 1, 17, 256), np.float32)
    xt[:, 0, :] = 1.0
    xt[:t_steps, 1:17, :] = np.asarray(x_core).transpose(1, 2, 0)
    xt[t_steps, 1:17, :] = 0.0
    return xt


_RUNNER_CACHE = {}


def _get_runner(t_steps):
    """Compile once; return fn(concat_inputs: dict name->global np array) -> y
    plus a bench fn that re-executes on device-resident inputs."""
    if t_steps in _RUNNER_CACHE:
        return _RUNNER_CACHE[t_steps]

    import jax
    from jax.experimental.shard_map import shard_map
    from jax.sharding import Mesh, NamedSharding, PartitionSpec

    from concourse import bass2jax

    bass2jax.install_neuronx_cc_hook()
    nc = build_nc(t_steps)
    if not nc.is_finalized():
        nc.finalize()
    global _LAST_NC
    _LAST_NC = nc

    partition_name = (
        nc.partition_id_tensor.name if nc.partition_id_tensor else None
    )
    in_names = []
    out_names = []
    out_avals = []
    zero_outs = []
    for alloc in nc.m.functions[0].allocations:
        if not isinstance(alloc, mybir.MemoryLocationSet):
            continue
        name = alloc.memorylocations[0].name
        if alloc.kind == "ExternalInput":
            if name == partition_name:
                continue
            in_names.append(name)
        elif alloc.kind == "ExternalOutput":
            out_names.append(name)
            shape = tuple(alloc.tensor_shape)
            dtype = mybir.dt.np(alloc.dtype)
            out_avals.append(jax.core.ShapedArray(shape, dtype))
            zero_outs.append(np.zeros(shape, dtype))
    n_params = len(in_names)
    all_in_names = in_names + out_names
    if partition_name is not None:
        all_in_names = all_in_names + [partition_name]

    def _body(*args):
        operands = list(args)
        if partition_name is not None:
            operands.append(bass2jax.partition_id_tensor())
        outs = bass2jax._bass_exec_p.bind(
            *operands,
            out_avals=tuple(out_avals),
            in_names=tuple(all_in_names),
            out_names=tuple(out_names),
            lowering_input_output_aliases=(),
            sim_require_finite=True,
            sim_require_nnan=True,
            nc=nc,
        )
        return tuple(outs)

    devices = jax.devices()[:N_CORES]
    mesh = Mesh(np.asarray(devices), ("core",))
    spec = PartitionSpec("core")
    in_specs = (spec,) * (n_params + len(out_names))
    out_specs = (spec,) * len(out_names)
    sharded = jax.jit(
        shard_map(_body, mesh=mesh, in_specs=in_specs, out_specs=out_specs,
                  check_rep=False),
        keep_unused=True,
    )
    sharding = NamedSharding(mesh, spec)

    def run(concat_inputs, n_bench=0):
        import time as _time

        args = [jax.device_put(concat_inputs[n], sharding) for n in in_names]
        args += [jax.device_put(
            np.zeros((N_CORES * z.shape[0], *z.shape[1:]), z.dtype), sharding)
            for z in zero_outs]
        outs = jax.block_until_ready(sharded(*args))
        bench_ns = None
        if n_bench:
            times = []
            for _ in range(n_bench):
                t0 = _time.perf_counter()
                jax.block_until_ready(sharded(*args))
                times.append(_time.perf_counter() - t0)
            bench_ns = int(min(times) * 1e9)
        y = np.asarray(outs[out_names.index("y")])
        return y, bench_ns

    _RUNNER_CACHE[t_steps] = run
    return run


def make_inputs(x, W_ih0, W_hh0, b_ih0, b_hh0, W_ih1, W_hh1, b_ih1, b_hh1,
                W_fc, b_fc):
    x = np.asarray(x, dtype=np.float32)
    t_steps = x.shape[1]
    cb = prep_weights(
        W_ih0, W_hh0, b_ih0, b_hh0, W_ih1, W_hh1, b_ih1, b_hh1, W_fc, b_fc
    )
    xt_all = np.empty((N_CORES * (t_steps + 1), 17, 256), np.float32)
    a0_all = np.zeros((N_CORES * 117, 256), np.float32)
    for core in range(N_CORES):
        xc = x[core * B_LOCAL : (core + 1) * B_LOCAL]
        xt = prep_x_core(xc, t_steps)
        xt_all[core * (t_steps + 1) : (core + 1) * (t_steps + 1)] = xt
        a0_all[core * 117 + 100 : (core + 1) * 117] = xt[0]
    reps = lambda a: np.concatenate([a] * N_CORES, axis=0)
    return t_steps, {
        "xt": xt_all,
        "cblob": reps(cb),
        "a0": a0_all,
    }


def kernel(x, W_ih0, W_hh0, b_ih0, b_hh0, W_ih1, W_hh1, b_ih1, b_hh1, W_fc, b_fc,
           n_bench=0):
    global LAST_EXEC_NS
    t_steps, concat_inputs = make_inputs(
        x, W_ih0, W_hh0, b_ih0, b_hh0, W_ih1, W_hh1, b_ih1, b_hh1, W_fc, b_fc
    )
    run = _get_runner(t_steps)
    y, bench_ns = run(concat_inputs, n_bench=n_bench)
    if bench_ns is not None:
        LAST_EXEC_NS = bench_ns
    return y.astype(np.float32)
